# revision 1
# baseline (speedup 1.0000x reference)
"""Trainium2 Bass kernel for nn_AttentionBlock (sparse bilinear attention).

Reference computation (N_NET=1, D=4, N_H=8, N_T=2048, N_IN=N_OUT=256):
    Omega[N,b,h,t,u] = r'[N,b,t,i] Q[N,h,i,j] r'[N,b,u,j]
    Omega *= tril(ones(T, T))                      # causal mask
    r[N,b,t,i] = Omega[N,b,h,t,u] E[N,h,i,j] r'[N,b,u,j]   # sums over h

Sharding across 8 NeuronCores: core c handles batch b = c//2 and the 4
heads [4*(c%2), 4*(c%2)+4). Each core produces the partial output for its
batch summed over its 4 heads; the host adds the two head-group partials.

Per-core device algorithm (contractions on the partition axis; float16
matmuls run at full PE rate with fast weight loads, ~3.5e-4 relative
error; float32r [~1.75e-4, ~13% slower] and bfloat16 remain selectable
via _build_nc(bf16=...)):
    AT[h](j,t)  = sum_i Q[h](i,j) rT(i,t)          (j on partitions)
    V[h](u,i)   = sum_j rT(j,u) ET[h](j,i)         (u on partitions,
                                                    two heads per matmul)
    S(u,t)      = sum_j rT(j,u) AT[h](j,t)         = Omega^T tile
    outT(i,t)  += sum_u V[h](u,i) S(u,t)           (PSUM-accumulated over
                                                    all heads & u-blocks)
Causality (keep u <= t): u-blocks entirely above the diagonal are skipped,
diagonal tiles are computed only on their nonzero column range with a
128x128 triangular mask applied to the crossing sub-block.

Main-loop S matmuls are emitted u-block-major with head-PAIRS sharing the
same stationary operand in consecutive matmuls (jc-clustered) — measured
substantially faster on hardware than head-major order (same results).
"""

import numpy as np

N_T = 2048
N_IN = 256
T_TILE = 512
TT = N_T // T_TILE  # 4 t-tiles
UB = N_T // 128     # 16 u-blocks
HL = 4              # heads per core
N_CORES = 8

_cache = {}


def _tri_mask():
    # mask[p, c] = 1 if c >= p  (keep u <= t on the diagonal sub-block)
    idx = np.arange(128)
    return (idx[None, :] >= idx[:, None]).astype(np.float32)


def _build_nc(repeat=1, bf16=False):
    import concourse.tile as tile
    import concourse.mybir as mybir
    from concourse import bacc

    F32 = mybir.dt.float32
    F32R = {False: mybir.dt.float32r, True: mybir.dt.bfloat16,
            "fp16": mybir.dt.float16}[bf16]

    nc = bacc.Bacc("TRN2", target_bir_lowering=False, debug=False,
                   num_devices=N_CORES)
    rT_d = nc.dram_tensor("rT", (2, 128, N_T), F32R, kind="ExternalInput").ap()
    Q4_d = nc.dram_tensor("Q4", (HL, 2, 128, N_IN), F32R,
                          kind="ExternalInput").ap()
    ET4_d = nc.dram_tensor("ET4", (HL, 2, 128, N_IN), F32R,
                           kind="ExternalInput").ap()
    mask_d = nc.dram_tensor("mask", (128, 128), F32R,
                            kind="ExternalInput").ap()
    outT_d = nc.dram_tensor("outT", (2, 128, N_T), F32,
                            kind="ExternalOutput").ap()

    # running per-engine copy-cost estimates for greedy DVE/ACT balancing
    eng_load = {"v": 0.0, "s": 0.0}

    def copy_psum(out_ap, in_ap, n):
        dve = n / 0.96 + 150.0
        act = (n + 352.0) / 1.2
        if eng_load["v"] + dve <= eng_load["s"] + act:
            eng_load["v"] += dve
            nc.vector.tensor_copy(out_ap, in_ap)
        else:
            eng_load["s"] += act
            nc.scalar.copy(out_ap, in_ap)

    with tile.TileContext(nc) as tc:
        with (
            tc.tile_pool(name="const", bufs=1) as const,
            tc.tile_pool(name="spool", bufs=8) as spool,
            tc.tile_pool(name="opool", bufs=4) as opool,
            tc.tile_pool(name="psum", bufs=5, space="PSUM") as psum,
            tc.tile_pool(name="psout", bufs=3, space="PSUM") as psout,
        ):
            # --- PE warm-up: dummy matmuls on memset data run during the
            # input-DMA lead-in so the HAM un-throttles (1.2->2.4 GHz)
            # before the first real matmul ---
            warm_f32 = const.tile([128, 128], F32)
            nc.vector.memset(warm_f32, 0.0)
            warm_sb = const.tile([128, 128], F32R)
            nc.vector.tensor_copy(warm_sb, warm_f32)
            warm_ps = psum.tile([128, T_TILE], F32, tag="ps", name="warm_ps")
            for _w in range(24):
                nc.tensor.matmul(warm_ps[:, :128], warm_sb, warm_sb,
                                 start=True, stop=True, skip_group_check=True)

            # --- inputs, finely tiled so compute can start ASAP ---
            mask_sb = const.tile([128, 128], F32R)
            Q_h = [const.tile([128, 2, N_IN], F32R, name=f"Qh{h}")
                   for h in range(HL)]
            rT_t = [[const.tile([128, T_TILE], F32R, name=f"rT{ic}_{tq}")
                     for tq in range(TT)] for ic in range(2)]
            ET_p = [const.tile([128, 2, 2, N_IN], F32R, name=f"ETp{p}")
                    for p in range(2)]
            for ic in range(2):
                nc.sync.dma_start(out=Q_h[0][:, ic, :], in_=Q4_d[0, ic])
            for tq in range(TT):
                for ic in range(2):
                    nc.sync.dma_start(
                        out=rT_t[ic][tq],
                        in_=rT_d[ic, :, T_TILE * tq:T_TILE * (tq + 1)])
            for hl in range(1, HL):
                for ic in range(2):
                    nc.sync.dma_start(out=Q_h[hl][:, ic, :], in_=Q4_d[hl, ic])
            for p2 in range(2):
                for jc in range(2):
                    for h2 in range(2):
                        nc.sync.dma_start(out=ET_p[p2][:, jc, h2, :],
                                          in_=ET4_d[2 * p2 + h2, jc])
            nc.sync.dma_start(out=mask_sb, in_=mask_d)

            # u-block ub lives in rT tile [ub // 4], columns 128*(ub % 4)
            def rT_ub(jc, ub):
                c0 = 128 * (ub % 4)
                return rT_t[jc][ub // 4][:, c0:c0 + 128]

            AT = [[[const.tile([128, T_TILE], F32R, name=f"AT{h}_{j}_{t}")
                    for t in range(TT)] for j in range(2)] for h in range(HL)]
            # V pair tiles: [p2][ub] -> (128, 2 heads, 256)
            Vp = [[const.tile([128, 2, N_IN], F32R, name=f"V{p}_{u}")
                   for u in range(UB)] for p in range(2)]

            def emit_a(hl, tts):
                for tt in tts:
                    for jc in range(2):
                        ps_a = psum.tile([128, T_TILE], F32, tag="ps",
                                         name="ps_a")
                        for ic in range(2):
                            nc.tensor.matmul(
                                ps_a,
                                Q_h[hl][:, ic, 128 * jc:128 * (jc + 1)],
                                rT_t[ic][tt],
                                start=(ic == 0), stop=(ic == 1))
                        copy_psum(AT[hl][jc][tt], ps_a, T_TILE)

            def emit_v(p2, ubs):
                for ub in ubs:
                    ps_v = psum.tile([128, 2, N_IN], F32, tag="ps",
                                     name="ps_v")
                    for jc in range(2):
                        nc.tensor.matmul(
                            ps_v, rT_ub(jc, ub), ET_p[p2][:, jc, :, :],
                            start=(jc == 0), stop=(jc == 1))
                    copy_psum(Vp[p2][ub], ps_v, 2 * N_IN)

            def body():
                # ---- Phase A (AT per head), then Phase V ----
                for hl in range(HL):
                    emit_a(hl, range(TT))
                for p2 in range(2):
                    emit_v(p2, range(UB))

                # ---- Main: S tiles + PSUM-accumulated output ----
                for tt in range(TT):
                    t0 = T_TILE * tt
                    po = [psout.tile([128, T_TILE], F32, tag="po",
                                     name=f"po{ic}") for ic in range(2)]
                    n_ub = 4 * tt + 4
                    for ub in range(n_ub):
                        d = ub - 4 * tt  # >=0: diagonal sub-block index
                        lo = max(d, 0) * 128
                        width = T_TILE - lo
                        # pairs of heads, jc-clustered so consecutive
                        # matmuls share the same stationary operand; each
                        # pair is drained (mask/copy + O-matmuls) before the
                        # next so only 2 S psum tiles are live at once
                        for hp in range(2):
                            pair = [2 * hp, 2 * hp + 1]
                            ps_pair = {hl: psum.tile([128, T_TILE], F32,
                                                     tag="ps",
                                                     name=f"ps_s{hl}")
                                       for hl in pair}
                            for jc in range(2):
                                w = rT_ub(jc, ub)
                                for hl in pair:
                                    nc.tensor.matmul(
                                        ps_pair[hl][:, lo:T_TILE],
                                        w,
                                        AT[hl][jc][tt][:, lo:T_TILE],
                                        start=(jc == 0), stop=(jc == 1))
                            for hl in pair:
                                ps_s = ps_pair[hl]
                                s_sb = spool.tile([128, T_TILE], F32R,
                                                  tag="s", name="s_sb")
                                if d >= 0:
                                    nc.vector.tensor_mul(
                                        s_sb[:, lo:lo + 128],
                                        ps_s[:, lo:lo + 128], mask_sb)
                                    eng_load["v"] += 128 / 0.96 + 150.0
                                    if width > 128:
                                        nc.vector.tensor_copy(
                                            s_sb[:, lo + 128:T_TILE],
                                            ps_s[:, lo + 128:T_TILE])
                                        eng_load["v"] += (width - 128) / 0.96 + 150.0
                                else:
                                    copy_psum(s_sb, ps_s, T_TILE)
                                first = (ub == 0 and hl == 0)
                                last = (ub == n_ub - 1 and hl == HL - 1)
                                for ic in range(2):
                                    nc.tensor.matmul(
                                        po[ic][:, lo:T_TILE],
                                        Vp[hl // 2][ub][:, hl % 2,
                                                        128 * ic:
                                                        128 * (ic + 1)],
                                        s_sb[:, lo:T_TILE],
                                        start=first, stop=last,
                                        skip_group_check=True)
                    for ic in range(2):
                        ot = opool.tile([128, T_TILE], F32, tag="ot",
                                        name="ot")
                        if ic == 0:
                            nc.vector.tensor_copy(ot, po[ic])
                        else:
                            nc.scalar.copy(ot, po[ic])
                        nc.sync.dma_start(
                            out=outT_d[ic, :, t0:t0 + T_TILE], in_=ot)

            if repeat == 1:
                body()
            elif repeat < 0:  # unrolled repeat (timing experiments)
                for _ in range(-repeat):
                    body()
            else:
                with tc.For_i(0, repeat, 1):
                    body()
    nc.compile()
    return nc


def _prep_in_maps(r_prime, E, Q, bf16=False):
    if bf16 == "fp16":
        cast_dt = np.float16
    elif bf16:
        import ml_dtypes
        cast_dt = ml_dtypes.bfloat16
    else:
        cast_dt = np.float32
    mask = _tri_mask()
    in_maps = []
    for c in range(N_CORES):
        b, hg = divmod(c, 2)
        heads = slice(4 * hg, 4 * hg + 4)
        rT = np.ascontiguousarray(r_prime[0, b].T).reshape(2, 128, N_T)
        Q4 = np.ascontiguousarray(Q[0, heads]).reshape(HL, 2, 128, N_IN)
        ET4 = np.ascontiguousarray(
            E[0, heads].transpose(0, 2, 1)).reshape(HL, 2, 128, N_IN)
        in_maps.append({"rT": rT.astype(cast_dt),
                        "Q4": Q4.astype(cast_dt),
                        "ET4": ET4.astype(cast_dt),
                        "mask": mask.astype(cast_dt)})
    return in_maps


DTYPE = "fp16"  # float16 matmuls: full PE rate + fast weight loads,
                # rel err ~3.5e-4 (float32r: 1.75e-4 but ~13% slower)


def kernel(r_prime, E, Q):
    from concourse import bass_utils

    if "nc" not in _cache:
        _cache["nc"] = _build_nc(bf16=DTYPE)
    nc = _cache["nc"]
    in_maps = _prep_in_maps(r_prime, E, Q, bf16=DTYPE)
    res = bass_utils.run_bass_kernel_spmd(nc, in_maps,
                                          core_ids=list(range(N_CORES)))
    out = np.zeros((1, 4, N_T, N_IN), dtype=np.float32)
    for b in range(4):
        acc = (res.results[2 * b]["outT"].reshape(N_IN, N_T)
               + res.results[2 * b + 1]["outT"].reshape(N_IN, N_T))
        out[0, b] = acc.T
    return out



# revision 5
# speedup vs baseline: 2.0512x; 2.0512x over previous
"""Trainium2 Bass kernel for nn_AttentionBlock (causal bilinear attention).

Reference computation (N_NET=1, D=4, N_H=8, N_T=2048, N_IN=N_OUT=256):
    Omega[N,b,h,t,u] = r'[N,b,t,i] Q[N,h,i,j] r'[N,b,u,j]
    Omega *= tril(ones(T, T))                      # causal mask (u <= t)
    r[N,b,t,i] = Omega[N,b,h,t,u] E[N,h,i,j] r'[N,b,u,j]   # sums over h

There is no softmax, so this is exact causal LINEAR attention and the
chunked prefix-state algorithm applies. With A_h = r' Q_h  [t, j] and
V_h[u] = E_h r'_u  [u, i], for chunk k of size C=128:

    out[t in k]  = sum_h A_h[t] @ P_h(k)          # inter-chunk (prefix state)
                 + sum_h sum_{u in k, u<=t} Omega[t,u] V_h[u]   # intra-chunk
    P_h(k+1)     = P_h(k) + r'[k-chunk]^T @ V_h[k-chunk]        # [j, i] state

This computes ~164k PE columns per core instead of ~344k for the direct
block-causal algorithm (the T x T score matrix never materializes beyond
one 128x128 diagonal block per head).

Sharding across 8 NeuronCores: core c handles batch b = c//2 and the 4
heads [4*(c%2), 4*(c%2)+4). Each core produces the partial output for its
batch summed over its 4 heads; the host adds the two head-group partials.

Per-core device algorithm (all matmuls fp16 with fp32 PSUM accumulate):
  Phase A:  A_sb[h][jc](j, t) = sum_i Q_h(i, j-blk) rT(i, t)   [j on parts]
  Chunk loop k = 0..15 (chunk = 128 positions):
    V(u,i)     = sum_j rT(j, u-chunk) ET_h(j, i)     (2 heads per matmul)
    S(u,t)     = sum_j rT(j, u-chunk) A_sb[h][jc](j, t-chunk)  = Omega^T
    s_sb       = S * tri_mask  (keep u <= t; DVE, fp16 out)
    apply:  po(t,i) += A_sb[h][jb](j, t-chunk)^T @ P_sb[h](j-blk, i)
    state:  P_ps[h](j,i) += rN(u-chunk, j-blk)^T @ V   (PSUM-resident prefix)
    intra:  po(t,i) += s_sb(u, t)^T @ V(u, i)
    P_sb[h] is the fp16 copy of P_ps[h] taken BEFORE this chunk's state
    update (so it holds the prefix over chunks < k).
Output is produced in natural [t, i] layout (no host transpose).
"""

import numpy as np

N_T = 2048
N_IN = 256
CH = 128            # chunk size
NCH = N_T // CH     # 16 chunks
T_TILE = 512
TT = N_T // T_TILE  # 4 t-tiles for phase A
HL = 4              # heads per core
N_CORES = 8

_cache = {}


def _tri_mask():
    # mask[u, t] = 1 if t >= u  (keep u <= t on the diagonal block)
    idx = np.arange(128)
    return (idx[None, :] >= idx[:, None]).astype(np.float32)


def _build_nc(repeat=1, bf16=False):
    import concourse.tile as tile
    import concourse.mybir as mybir
    from concourse import bacc

    F32 = mybir.dt.float32
    F16 = {False: mybir.dt.float32r, True: mybir.dt.bfloat16,
           "fp16": mybir.dt.float16}[bf16]

    nc = bacc.Bacc("TRN2", target_bir_lowering=False, debug=False,
                   num_devices=N_CORES)
    rT_d = nc.dram_tensor("rT", (2, 128, N_T), F16, kind="ExternalInput").ap()
    rN_d = nc.dram_tensor("rN", (128, NCH, N_IN), F16,
                          kind="ExternalInput").ap()
    Q4_d = nc.dram_tensor("Q4", (HL, 2, 128, N_IN), F16,
                          kind="ExternalInput").ap()
    ET4_d = nc.dram_tensor("ET4", (HL, 2, 128, N_IN), F16,
                           kind="ExternalInput").ap()
    mask_d = nc.dram_tensor("mask", (128, 128), F16,
                            kind="ExternalInput").ap()
    out_d = nc.dram_tensor("out", (N_T, N_IN), F32,
                           kind="ExternalOutput").ap()

    # running per-engine cost estimates for greedy DVE/ACT balancing
    eng_load = {"v": 0.0, "s": 0.0}

    def copy_psum(out_ap, in_ap, n):
        dve = n / 0.96 + 150.0
        act = (n + 352.0) / 1.2
        if eng_load["v"] + dve <= eng_load["s"] + act:
            eng_load["v"] += dve
            nc.vector.tensor_copy(out_ap, in_ap)
        else:
            eng_load["s"] += act
            nc.scalar.copy(out_ap, in_ap)

    with tile.TileContext(nc) as tc:
        with (
            tc.tile_pool(name="const", bufs=1) as const,
            tc.tile_pool(name="vpool", bufs=4) as vpool,
            tc.tile_pool(name="spool", bufs=8) as spool,
            tc.tile_pool(name="ppool", bufs=8) as ppool,
            tc.tile_pool(name="opool", bufs=3) as opool,
            tc.tile_pool(name="psum", bufs=3, space="PSUM") as psum,
            tc.tile_pool(name="pstate", bufs=4, space="PSUM") as pstate,
            tc.tile_pool(name="psout", bufs=1, space="PSUM") as psout,
        ):
            # --- PE warm-up: dummy matmuls on memset data run during the
            # input-DMA lead-in so the HAM un-throttles (1.2->2.4 GHz) ---
            warm_f32 = const.tile([128, 128], F32)
            nc.vector.memset(warm_f32, 0.0)
            warm_sb = const.tile([128, 128], F16)
            nc.vector.tensor_copy(warm_sb, warm_f32)
            warm_ps = psum.tile([128, T_TILE], F32, tag="ps", name="warm_ps")
            for _w in range(24):
                nc.tensor.matmul(warm_ps[:, :128], warm_sb, warm_sb,
                                 start=True, stop=True, skip_group_check=True)

            # --- inputs ---
            mask_sb = const.tile([128, 128], F16)
            Q_h = [const.tile([128, 2, N_IN], F16, name=f"Qh{h}")
                   for h in range(HL)]
            rT_t = [[const.tile([128, T_TILE], F16, name=f"rT{ic}_{tq}")
                     for tq in range(TT)] for ic in range(2)]
            rN_sb = const.tile([128, NCH, N_IN], F16, name="rN")
            ET_p = [const.tile([128, 2, 2, N_IN], F16, name=f"ETp{p}")
                    for p in range(2)]
            for ic in range(2):
                nc.sync.dma_start(out=Q_h[0][:, ic, :], in_=Q4_d[0, ic])
            for tq in range(TT):
                for ic in range(2):
                    nc.sync.dma_start(
                        out=rT_t[ic][tq],
                        in_=rT_d[ic, :, T_TILE * tq:T_TILE * (tq + 1)])
            for hl in range(1, HL):
                for ic in range(2):
                    nc.sync.dma_start(out=Q_h[hl][:, ic, :], in_=Q4_d[hl, ic])
            nc.sync.dma_start(out=rN_sb, in_=rN_d)
            for p2 in range(2):
                for jc in range(2):
                    for h2 in range(2):
                        nc.sync.dma_start(out=ET_p[p2][:, jc, h2, :],
                                          in_=ET4_d[2 * p2 + h2, jc])
            nc.sync.dma_start(out=mask_sb, in_=mask_d)

            # chunk k of rT lives in tile [k // 4], columns 128*(k % 4)
            def rT_ch(jc, k):
                c0 = 128 * (k % 4)
                return rT_t[jc][k // 4][:, c0:c0 + 128]

            A_sb = [[const.tile([128, N_T], F16, name=f"A{h}_{j}")
                     for j in range(2)] for h in range(HL)]

            def body():
                # ---- Phase A: A_sb[h][jc](j, t) for all t ----
                for tq in range(TT):
                    for hl in range(HL):
                        for jc in range(2):
                            ps_a = psum.tile([128, T_TILE], F32, tag="ps",
                                             name="ps_a")
                            for ic in range(2):
                                nc.tensor.matmul(
                                    ps_a,
                                    Q_h[hl][:, ic, 128 * jc:128 * (jc + 1)],
                                    rT_t[ic][tq],
                                    start=(ic == 0), stop=(ic == 1))
                            copy_psum(
                                A_sb[hl][jc][:, T_TILE * tq:T_TILE * (tq + 1)],
                                ps_a, T_TILE)

                # ---- persistent per-head state PSUM (prefix over chunks) ---
                P_ps = [pstate.tile([128, 2, N_IN], F32, tag="pp",
                                    name=f"P{h}") for h in range(HL)]

                # ---- chunk loop ----
                for k in range(NCH):
                    t0 = CH * k
                    # S scores for all 4 heads (one PSUM bank, jc-clustered)
                    ps_s = psum.tile([128, HL, CH], F32, tag="ps",
                                     name="ps_s")
                    # NOTE: PSUM 'start' zeroes the whole 2KB bank, so only
                    # the first matmul into the bank may set it; the other
                    # head sub-groups accumulate into the zeroed region.
                    for jc in range(2):
                        w = rT_ch(jc, k)
                        for hl in range(HL):
                            nc.tensor.matmul(
                                ps_s[:, hl, :], w,
                                A_sb[hl][jc][:, t0:t0 + CH],
                                start=(jc == 0 and hl == 0),
                                stop=(jc == 1 and hl == HL - 1),
                                skip_group_check=True)
                    # V for this chunk (2 heads per matmul)
                    vt = []
                    for p2 in range(2):
                        ps_v = psum.tile([128, 2, N_IN], F32, tag="ps",
                                         name="ps_v")
                        for jc in range(2):
                            nc.tensor.matmul(ps_v, rT_ch(jc, k),
                                             ET_p[p2][:, jc, :, :],
                                             start=(jc == 0), stop=(jc == 1))
                        v_sb = vpool.tile([128, 2, N_IN], F16, tag="v",
                                          name="v_sb")
                        copy_psum(v_sb, ps_v, 2 * N_IN)
                        vt.append(v_sb)
                    # prefix-state copies for the apply step (chunks < k)
                    P_sb = None
                    if k >= 1:
                        P_sb = [ppool.tile([128, 2, N_IN], F16, tag="p",
                                           name=f"P_sb{h}")
                                for h in range(HL)]
                        for h in range(HL):
                            copy_psum(P_sb[h], P_ps[h], 2 * N_IN)
                    # masked scores -> fp16 (DVE only: tensor_mul)
                    s_sb = []
                    for hl in range(HL):
                        s = spool.tile([128, CH], F16, tag="s", name="s_sb")
                        nc.vector.tensor_mul(s, ps_s[:, hl, :], mask_sb)
                        eng_load["v"] += CH / 0.96 + 150.0
                        s_sb.append(s)

                    po = psout.tile([128, N_IN], F32, tag="po", name="po")
                    # inter-chunk apply: po(t,i) += A^T P  (8 matmuls)
                    if k >= 1:
                        for hl in range(HL):
                            for jb in range(2):
                                nc.tensor.matmul(
                                    po, A_sb[hl][jb][:, t0:t0 + CH],
                                    P_sb[hl][:, jb, :],
                                    start=(hl == 0 and jb == 0), stop=False,
                                    skip_group_check=True)
                    # state update: P_ps[h] += rN^T V  (8 matmuls, k<15)
                    if k < NCH - 1:
                        for hl in range(HL):
                            for jb in range(2):
                                nc.tensor.matmul(
                                    P_ps[hl][:, jb, :],
                                    rN_sb[:, k, 128 * jb:128 * (jb + 1)],
                                    vt[hl // 2][:, hl % 2, :],
                                    start=(k == 0 and jb == 0),
                                    stop=(k == NCH - 2 and jb == 1),
                                    skip_group_check=True)
                    # intra-chunk: po(t,i) += s_sb^T V  (4 matmuls)
                    for hl in range(HL):
                        nc.tensor.matmul(
                            po, s_sb[hl], vt[hl // 2][:, hl % 2, :],
                            start=(k == 0 and hl == 0), stop=(hl == HL - 1),
                            skip_group_check=True)
                    # drain output chunk
                    ot = opool.tile([128, N_IN], F32, tag="ot", name="ot")
                    copy_psum(ot, po, N_IN)
                    nc.sync.dma_start(out=out_d[t0:t0 + CH, :], in_=ot)

            if repeat == 1:
                body()
            elif repeat < 0:  # unrolled repeat (timing experiments)
                for _ in range(-repeat):
                    body()
            else:
                with tc.For_i(0, repeat, 1):
                    body()
    nc.compile()
    return nc


def _prep_in_maps(r_prime, E, Q, bf16=False):
    if bf16 == "fp16":
        cast_dt = np.float16
    elif bf16:
        import ml_dtypes
        cast_dt = ml_dtypes.bfloat16
    else:
        cast_dt = np.float32
    mask = _tri_mask()
    in_maps = []
    for c in range(N_CORES):
        b, hg = divmod(c, 2)
        heads = slice(4 * hg, 4 * hg + 4)
        rb = r_prime[0, b]                       # [T, I]
        rT = np.ascontiguousarray(rb.T).reshape(2, 128, N_T)
        rN = np.ascontiguousarray(
            rb.reshape(NCH, 128, N_IN).transpose(1, 0, 2))  # [u%128, k, j]
        Q4 = np.ascontiguousarray(Q[0, heads]).reshape(HL, 2, 128, N_IN)
        ET4 = np.ascontiguousarray(
            E[0, heads].transpose(0, 2, 1)).reshape(HL, 2, 128, N_IN)
        in_maps.append({"rT": rT.astype(cast_dt),
                        "rN": rN.astype(cast_dt),
                        "Q4": Q4.astype(cast_dt),
                        "ET4": ET4.astype(cast_dt),
                        "mask": mask.astype(cast_dt)})
    return in_maps


DTYPE = "fp16"  # float16 matmuls: full PE rate + fast weight loads


def kernel(r_prime, E, Q):
    from concourse import bass_utils

    if "nc" not in _cache:
        _cache["nc"] = _build_nc(bf16=DTYPE)
    nc = _cache["nc"]
    in_maps = _prep_in_maps(r_prime, E, Q, bf16=DTYPE)
    res = bass_utils.run_bass_kernel_spmd(nc, in_maps,
                                          core_ids=list(range(N_CORES)))
    out = np.zeros((1, 4, N_T, N_IN), dtype=np.float32)
    for b in range(4):
        out[0, b] = res.results[2 * b]["out"] + res.results[2 * b + 1]["out"]
    return out


# revision 7
# speedup vs baseline: 3.8454x; 1.8747x over previous
"""Trainium2 Bass kernel for nn_AttentionBlock (causal bilinear attention).

Reference computation (N_NET=1, D=4, N_H=8, N_T=2048, N_IN=N_OUT=256):
    Omega[N,b,h,t,u] = r'[N,b,t,i] Q[N,h,i,j] r'[N,b,u,j]
    Omega *= tril(ones(T, T))                      # causal mask (u <= t)
    r[N,b,t,i] = Omega[N,b,h,t,u] E[N,h,i,j] r'[N,b,u,j]   # sums over h

There is no softmax, so this is exact causal LINEAR attention and the
chunked prefix-state algorithm applies. With A_h = r' Q_h  [t, j] and
V_h[u] = E_h r'_u  [u, i], for chunk k of size C=128:

    out[t in k]  = sum_h A_h[t] @ P_h(k)          # inter-chunk (prefix state)
                 + sum_h sum_{u in k, u<=t} Omega[t,u] V_h[u]   # intra-chunk
    P_h(k+1)     = P_h(k) + r'[k-chunk]^T @ V_h[k-chunk]        # [j, i] state

This computes ~164k PE columns per core instead of ~344k for the direct
block-causal algorithm (the T x T score matrix never materializes beyond
one 128x128 diagonal block per head).

Sharding across 8 NeuronCores: core c handles batch b = c//2 and the 4
heads [4*(c%2), 4*(c%2)+4). Each core produces the partial output for its
batch summed over its 4 heads; the host adds the two head-group partials.

Per-core device algorithm (all matmuls fp16 with fp32 PSUM accumulate):
  Phase A:  A_sb[h][jc](j, t) = sum_i Q_h(i, j-blk) rT(i, t)   [j on parts]
  Chunk loop k = 0..15 (chunk = 128 positions):
    V(u,i)     = sum_j rT(j, u-chunk) ET_h(j, i)     (2 heads per matmul)
    S(u,t)     = sum_j rT(j, u-chunk) A_sb[h][jc](j, t-chunk)  = Omega^T
    s_sb       = S * tri_mask  (keep u <= t; DVE, fp16 out)
    apply:  po(t,i) += A_sb[h][jb](j, t-chunk)^T @ P_sb[h](j-blk, i)
    state:  P_ps[h](j,i) += rN(u-chunk, j-blk)^T @ V   (PSUM-resident prefix)
    intra:  po(t,i) += s_sb(u, t)^T @ V(u, i)
    P_sb[h] is the fp16 copy of P_ps[h] taken BEFORE this chunk's state
    update (so it holds the prefix over chunks < k).
Output is produced in natural [t, i] layout (no host transpose).
"""

import numpy as np

N_T = 2048
N_IN = 256
CH = 128            # chunk size
NCH = N_T // CH     # 16 chunks
T_TILE = 512
TT = N_T // T_TILE  # 4 t-tiles for phase A
HL = 4              # heads per core
N_CORES = 8

_cache = {}


def _tri_mask():
    # mask[u, t] = 1 if t >= u  (keep u <= t on the diagonal block)
    idx = np.arange(128)
    return (idx[None, :] >= idx[:, None]).astype(np.float32)


def _build_nc(repeat=1, bf16=False):
    import concourse.tile as tile
    import concourse.mybir as mybir
    from concourse import bacc

    F32 = mybir.dt.float32
    F16 = {False: mybir.dt.float32r, True: mybir.dt.bfloat16,
           "fp16": mybir.dt.float16}[bf16]

    nc = bacc.Bacc("TRN2", target_bir_lowering=False, debug=False,
                   num_devices=N_CORES)
    rT_d = nc.dram_tensor("rT", (2, 128, N_T), F16, kind="ExternalInput").ap()
    rN_d = nc.dram_tensor("rN", (128, NCH, N_IN), F16,
                          kind="ExternalInput").ap()
    Q4_d = nc.dram_tensor("Q4", (HL, 2, 128, N_IN), F16,
                          kind="ExternalInput").ap()
    ET4_d = nc.dram_tensor("ET4", (HL, 2, 128, N_IN), F16,
                           kind="ExternalInput").ap()
    mask_d = nc.dram_tensor("mask", (128, 128), F16,
                            kind="ExternalInput").ap()
    out_d = nc.dram_tensor("out", (N_T, N_IN), F32,
                           kind="ExternalOutput").ap()

    # running per-engine cost estimates for greedy DVE/ACT balancing
    eng_load = {"v": 0.0, "s": 0.0}

    def copy_psum(out_ap, in_ap, n):
        dve = n / 0.96 + 150.0
        act = (n + 352.0) / 1.2
        if eng_load["v"] + dve <= eng_load["s"] + act:
            eng_load["v"] += dve
            nc.vector.tensor_copy(out_ap, in_ap)
        else:
            eng_load["s"] += act
            nc.scalar.copy(out_ap, in_ap)

    with tile.TileContext(nc) as tc:
        with (
            tc.tile_pool(name="const", bufs=1) as const,
            tc.tile_pool(name="vpool", bufs=4) as vpool,
            tc.tile_pool(name="spool", bufs=8) as spool,
            tc.tile_pool(name="ppool", bufs=8) as ppool,
            tc.tile_pool(name="opool", bufs=3) as opool,
            tc.tile_pool(name="psum", bufs=3, space="PSUM") as psum,
            tc.tile_pool(name="pstate", bufs=4, space="PSUM") as pstate,
            tc.tile_pool(name="psout", bufs=1, space="PSUM") as psout,
        ):
            # --- PE warm-up: dummy matmuls on memset data run during the
            # input-DMA lead-in so the HAM un-throttles (1.2->2.4 GHz) ---
            warm_f32 = const.tile([128, 128], F32)
            nc.vector.memset(warm_f32, 0.0)
            warm_sb = const.tile([128, 128], F16)
            nc.vector.tensor_copy(warm_sb, warm_f32)
            warm_ps = psum.tile([128, T_TILE], F32, tag="ps", name="warm_ps")
            for _w in range(24):
                nc.tensor.matmul(warm_ps[:, :128], warm_sb, warm_sb,
                                 start=True, stop=True, skip_group_check=True)

            # --- inputs ---
            mask_sb = const.tile([128, 128], F16)
            Q_h = [const.tile([128, 2, N_IN], F16, name=f"Qh{h}")
                   for h in range(HL)]
            rT_t = [[const.tile([128, T_TILE], F16, name=f"rT{ic}_{tq}")
                     for tq in range(TT)] for ic in range(2)]
            rN_sb = const.tile([128, NCH, N_IN], F16, name="rN")
            ET_p = [const.tile([128, 2, 2, N_IN], F16, name=f"ETp{p}")
                    for p in range(2)]
            for ic in range(2):
                nc.sync.dma_start(out=Q_h[0][:, ic, :], in_=Q4_d[0, ic])
            for tq in range(TT):
                for ic in range(2):
                    nc.sync.dma_start(
                        out=rT_t[ic][tq],
                        in_=rT_d[ic, :, T_TILE * tq:T_TILE * (tq + 1)])
            for hl in range(1, HL):
                for ic in range(2):
                    nc.sync.dma_start(out=Q_h[hl][:, ic, :], in_=Q4_d[hl, ic])
            nc.sync.dma_start(out=rN_sb, in_=rN_d)
            for p2 in range(2):
                for jc in range(2):
                    for h2 in range(2):
                        nc.sync.dma_start(out=ET_p[p2][:, jc, h2, :],
                                          in_=ET4_d[2 * p2 + h2, jc])
            nc.sync.dma_start(out=mask_sb, in_=mask_d)

            # chunk k of rT lives in tile [k // 4], columns 128*(k % 4)
            def rT_ch(jc, k):
                c0 = 128 * (k % 4)
                return rT_t[jc][k // 4][:, c0:c0 + 128]

            A_sb = [[const.tile([128, N_T], F16, name=f"A{h}_{j}")
                     for j in range(2)] for h in range(HL)]

            def body():
                # ---- Phase A: A_sb[h][jc](j, t) for all t ----
                # tq-pairs with ic outer so each Q stationary serves two
                # 512-col matmuls before reloading (half the weight loads).
                for hl in range(HL):
                    for jc in range(2):
                        for tp in range(TT // 2):
                            ps_p = [psum.tile([128, T_TILE], F32, tag="ps",
                                              name="ps_a") for _ in range(2)]
                            for ic in range(2):
                                for tq2 in range(2):
                                    nc.tensor.matmul(
                                        ps_p[tq2],
                                        Q_h[hl][:, ic,
                                                128 * jc:128 * (jc + 1)],
                                        rT_t[ic][2 * tp + tq2],
                                        start=(ic == 0), stop=(ic == 1))
                            for tq2 in range(2):
                                tq = 2 * tp + tq2
                                copy_psum(
                                    A_sb[hl][jc][:,
                                                 T_TILE * tq:T_TILE * (tq + 1)],
                                    ps_p[tq2], T_TILE)

                # ---- persistent state PSUM, keyed [p2][jb]; each bank holds
                # P[j in jb-block, h2, i] for the two heads of pair p2 ----
                P_ps = [[pstate.tile([128, 2, N_IN], F32, tag="pp",
                                     name=f"P{p2}_{jb}") for jb in range(2)]
                        for p2 in range(2)]

                # ---- chunk loop ----
                for k in range(NCH):
                    t0 = CH * k
                    # S scores (4 heads, one bank) + V (2 heads per matmul),
                    # jc-outer so one rT stationary serves 6 matmuls.
                    # NOTE: PSUM 'start' zeroes the whole 2KB bank, so only
                    # the first matmul into each bank may set it; later
                    # sub-groups accumulate into the zeroed region.
                    ps_s = psum.tile([128, HL, CH], F32, tag="ps",
                                     name="ps_s")
                    ps_v = [psum.tile([128, 2, N_IN], F32, tag="ps",
                                      name=f"ps_v{p2}") for p2 in range(2)]
                    for jc in range(2):
                        w = rT_ch(jc, k)
                        for hl in range(HL):
                            nc.tensor.matmul(
                                ps_s[:, hl, :], w,
                                A_sb[hl][jc][:, t0:t0 + CH],
                                start=(jc == 0 and hl == 0),
                                stop=(jc == 1 and hl == HL - 1),
                                skip_group_check=True)
                        for p2 in range(2):
                            nc.tensor.matmul(ps_v[p2], w,
                                             ET_p[p2][:, jc, :, :],
                                             start=(jc == 0), stop=(jc == 1))
                    vt = []
                    for p2 in range(2):
                        v_sb = vpool.tile([128, 2, N_IN], F16, tag="v",
                                          name="v_sb")
                        copy_psum(v_sb, ps_v[p2], 2 * N_IN)
                        vt.append(v_sb)
                    # prefix-state copies for the apply step (chunks < k)
                    P_sb = None
                    if k >= 1:
                        P_sb = [[ppool.tile([128, 2, N_IN], F16, tag="p",
                                            name=f"P_sb{p2}_{jb}")
                                 for jb in range(2)] for p2 in range(2)]
                        for p2 in range(2):
                            for jb in range(2):
                                copy_psum(P_sb[p2][jb], P_ps[p2][jb],
                                          2 * N_IN)
                    # masked scores -> fp16 (DVE only: tensor_mul)
                    s_sb = []
                    for hl in range(HL):
                        s = spool.tile([128, CH], F16, tag="s", name="s_sb")
                        nc.vector.tensor_mul(s, ps_s[:, hl, :], mask_sb)
                        eng_load["v"] += CH / 0.96 + 150.0
                        s_sb.append(s)

                    po = psout.tile([128, N_IN], F32, tag="po", name="po")
                    # inter-chunk apply: po(t,i) += A^T P  (8 matmuls)
                    if k >= 1:
                        for hl in range(HL):
                            for jb in range(2):
                                nc.tensor.matmul(
                                    po, A_sb[hl][jb][:, t0:t0 + CH],
                                    P_sb[hl // 2][jb][:, hl % 2, :],
                                    start=(hl == 0 and jb == 0), stop=False,
                                    skip_group_check=True)
                    # state update: P[p2][jb] += rN^T V, both heads per
                    # matmul; jb-outer so one rN stationary serves 2 matmuls
                    if k < NCH - 1:
                        for jb in range(2):
                            for p2 in range(2):
                                nc.tensor.matmul(
                                    P_ps[p2][jb],
                                    rN_sb[:, k, 128 * jb:128 * (jb + 1)],
                                    vt[p2],
                                    start=(k == 0), stop=(k == NCH - 2),
                                    skip_group_check=True)
                    # intra-chunk: po(t,i) += s_sb^T V  (4 matmuls)
                    for hl in range(HL):
                        nc.tensor.matmul(
                            po, s_sb[hl], vt[hl // 2][:, hl % 2, :],
                            start=(k == 0 and hl == 0), stop=(hl == HL - 1),
                            skip_group_check=True)
                    # drain output chunk
                    ot = opool.tile([128, N_IN], F32, tag="ot", name="ot")
                    copy_psum(ot, po, N_IN)
                    nc.sync.dma_start(out=out_d[t0:t0 + CH, :], in_=ot)

            if repeat == 1:
                body()
            elif repeat < 0:  # unrolled repeat (timing experiments)
                for _ in range(-repeat):
                    body()
            else:
                with tc.For_i(0, repeat, 1):
                    body()
    nc.compile()
    return nc


def _prep_in_maps(r_prime, E, Q, bf16=False):
    if bf16 == "fp16":
        cast_dt = np.float16
    elif bf16:
        import ml_dtypes
        cast_dt = ml_dtypes.bfloat16
    else:
        cast_dt = np.float32
    mask = _tri_mask()
    in_maps = []
    for c in range(N_CORES):
        b, hg = divmod(c, 2)
        heads = slice(4 * hg, 4 * hg + 4)
        rb = r_prime[0, b]                       # [T, I]
        rT = np.ascontiguousarray(rb.T).reshape(2, 128, N_T)
        rN = np.ascontiguousarray(
            rb.reshape(NCH, 128, N_IN).transpose(1, 0, 2))  # [u%128, k, j]
        Q4 = np.ascontiguousarray(Q[0, heads]).reshape(HL, 2, 128, N_IN)
        ET4 = np.ascontiguousarray(
            E[0, heads].transpose(0, 2, 1)).reshape(HL, 2, 128, N_IN)
        in_maps.append({"rT": rT.astype(cast_dt),
                        "rN": rN.astype(cast_dt),
                        "Q4": Q4.astype(cast_dt),
                        "ET4": ET4.astype(cast_dt),
                        "mask": mask.astype(cast_dt)})
    return in_maps


DTYPE = "fp16"  # float16 matmuls: full PE rate + fast weight loads


def kernel(r_prime, E, Q):
    from concourse import bass_utils

    if "nc" not in _cache:
        _cache["nc"] = _build_nc(bf16=DTYPE)
    nc = _cache["nc"]
    in_maps = _prep_in_maps(r_prime, E, Q, bf16=DTYPE)
    res = bass_utils.run_bass_kernel_spmd(nc, in_maps,
                                          core_ids=list(range(N_CORES)))
    out = np.zeros((1, 4, N_T, N_IN), dtype=np.float32)
    for b in range(4):
        out[0, b] = res.results[2 * b]["out"] + res.results[2 * b + 1]["out"]
    return out
